# revision 23
# baseline (speedup 1.0000x reference)
"""Trainium2 Bass kernel for nn_ConnectLoss.

loss = sum(relu(|x[:,j] - x[:,j-1]| - 1) * mask[:,j]) over j in [1, L).

Pure data-parallel over 8 NeuronCores: rows sharded 8192/core. Per
core, megatiles of 8x128 rows ([128, 8, 512] SBUF tiles) stream in on
two HWDGE queues (x via sync, mask via scalar, each 2-way split for
pacing); per megatile:
  DVE  tensor_tensor              d = x[:,1:] - x[:,:-1]
  ACT  activation(Abs, in-place)  d = |d|
  ACT  activation(Relu, bias=-1, in-place)  d = relu(d - 1)
  DVE  scalar_tensor_tensor       (d*1)*m with accum_out -> acc[:,t]
The kernel is DMA-bound (~33.5 MB/core streams at ~385 GB/s); the DVE
(~40%) and ACT (~30%) loads fit underneath. Host sums the 8 x
[128, n_mega] partials in float64.
"""
import sys

sys.path.insert(0, "/opt/trn_rl_repo")
import numpy as np

N_CORES = 8
M_ROWS = 65536
LENGTH = 512
ROWS_PER_CORE = M_ROWS // N_CORES
P = 128
BLOCKS = 8  # 128-row blocks fused per megatile (one DMA / DVE op each)

_nc_cache = None


def _build_nc(rows=ROWS_PER_CORE, length=LENGTH, blocks=BLOCKS):
    import concourse.tile as tile
    import concourse.mybir as mybir
    from concourse import bacc

    total_blocks = rows // P
    assert rows == total_blocks * P
    # Uniform megatiles (tapering measured slower).
    tail = []
    if total_blocks >= blocks + sum(tail):
        mid = total_blocks - sum(tail)
        schedule = [blocks] * (mid // blocks)
        rem = mid - (mid // blocks) * blocks
        if rem:
            schedule.append(rem)
        schedule += tail
    else:
        schedule = [blocks] * (total_blocks // blocks)
    n_mega = len(schedule)

    nc = bacc.Bacc(None)
    f32 = mybir.dt.float32
    x = nc.declare_dram_parameter("x", [rows, length], f32, isOutput=False)
    msk = nc.declare_dram_parameter("mask", [rows, length], f32, isOutput=False)
    out = nc.declare_dram_parameter("out", [P, n_mega], f32, isOutput=True)

    L1 = length - 1
    with tile.TileContext(nc) as tc:
        with (
            tc.tile_pool(name="xin", bufs=4) as xpool,
            tc.tile_pool(name="min", bufs=4) as mpool,
            tc.tile_pool(name="work", bufs=2) as wpool,
            tc.tile_pool(name="junk", bufs=1) as jpool,
            tc.tile_pool(name="acc", bufs=1) as apool,
        ):
            neg1 = apool.tile([P, 1], f32, tag="neg1")
            nc.vector.memset(neg1[:], -1.0)
            acc = apool.tile([P, n_mega], f32, tag="acc")
            junk = jpool.tile([P, blocks, length], f32, tag="junk")
            r0 = 0
            for t, nb in enumerate(schedule):
                xs = x[r0 * P : (r0 + nb) * P, :].rearrange(
                    "(b p) m -> p b m", p=P
                )
                ms = msk[r0 * P : (r0 + nb) * P, :].rearrange(
                    "(b p) m -> p b m", p=P
                )
                r0 += nb
                xt = xpool.tile([P, nb, length], f32, tag="xt")
                mt = mpool.tile([P, nb, length], f32, tag="mt")
                if nb > 1:
                    h = nb // 2
                    nc.sync.dma_start(xt[:, 0:h], xs[:, 0:h])
                    nc.sync.dma_start(xt[:, h:nb], xs[:, h:nb])
                    nc.scalar.dma_start(mt[:, 0:h], ms[:, 0:h])
                    nc.scalar.dma_start(mt[:, h:nb], ms[:, h:nb])
                else:
                    nc.sync.dma_start(xt[:], xs[:])
                    nc.scalar.dma_start(mt[:], ms[:])
                d = wpool.tile([P, nb, length], f32, tag="d")
                nc.vector.tensor_tensor(
                    d[:, :, 1:length],
                    xt[:, :, 1:length],
                    xt[:, :, 0:L1],
                    mybir.AluOpType.subtract,
                )
                nc.scalar.activation(
                    d[:, :, 1:length],
                    d[:, :, 1:length],
                    mybir.ActivationFunctionType.Abs,
                )
                nc.scalar.activation(
                    d[:, :, 1:length],
                    d[:, :, 1:length],
                    mybir.ActivationFunctionType.Relu,
                    bias=neg1[:, 0:1],
                    scale=1.0,
                )
                nc.vector.scalar_tensor_tensor(
                    junk[:, 0:nb, 1:length],
                    d[:, :, 1:length],
                    1.0,
                    mt[:, :, 1:length],
                    op0=mybir.AluOpType.mult,
                    op1=mybir.AluOpType.mult,
                    accum_out=acc[:, t : t + 1],
                )
            nc.sync.dma_start(out[:], acc[:])
    nc.compile()
    return nc


def _get_nc():
    global _nc_cache
    if _nc_cache is None:
        _nc_cache = _build_nc()
    return _nc_cache


def _finish(outs) -> np.ndarray:
    o = np.stack(outs).astype(np.float64)
    return np.asarray(o.sum(), dtype=np.float32)


def run_spmd(x, mask, trace: bool = False):
    """Returns (loss ndarray, BassKernelResults)."""
    from concourse.bass_utils import run_bass_kernel_spmd

    x = np.ascontiguousarray(np.asarray(x, dtype=np.float32))
    mask = np.ascontiguousarray(np.asarray(mask, dtype=np.float32))
    assert x.shape == (M_ROWS, LENGTH) and mask.shape == (M_ROWS, LENGTH)

    in_maps = [
        {
            "x": x[i * ROWS_PER_CORE : (i + 1) * ROWS_PER_CORE],
            "mask": mask[i * ROWS_PER_CORE : (i + 1) * ROWS_PER_CORE],
        }
        for i in range(N_CORES)
    ]
    res = run_bass_kernel_spmd(
        _get_nc(), in_maps, list(range(N_CORES)), trace=trace
    )
    loss = _finish([r["out"] for r in res.results])
    return loss, res


def kernel(x, mask) -> np.ndarray:
    loss, _ = run_spmd(x, mask, trace=False)
    return loss
